# revision 13
# baseline (speedup 1.0000x reference)
"""Attention-LSTM decoder (B=32, T=1000, S=100, D=512, A=1024, H=1024,
E=640, V=10240, P=1024) on 8 trn2 NeuronCores.

Sharding: data-parallel over batch, 4 batches per core (one per "slot").
Batches are sorted by enc_seq_len; slot j holds ranks [j*8:(j+1)*8] so the
padded time extent Tp[j] (multiple of 128) is shared by all 8 cores and the
SPMD graph is identical across cores while skipping work beyond each slot's
padded length.

v2 restructure vs baseline:
  - encoder tiles resident in SBUF (no per-step enc DMA reload)
  - e_t (precomputed enc context) stored fp8e4m3 to make SBUF room
  - all-tanh LSTM (sigmoid(x) = (1+tanh(x/2))/2 folded into the zoneout
    constants) so tanh+exp share one ACT table set -> zero per-step
    ACT_TABLE_LOADs
  - energies accumulate straight into a [4,1024] PSUM tile (out-partition
    = slot) -> softmax reads PSUM; no per-slot staging copies/DMAs
  - multiplicative 0/1 mask fused into the post-exp normalization
  - program order: gates -> LSTM -> s_t -> attention (z/tanh/energy MMs)
    -> W_hh for next step -> softmax -> q broadcast -> transposes -> ctx,
    so the PE never blocks the z/tanh pipeline and DVE z-ops overlap the
    gates phase of the next step
  - deep z/tv pools (4 bufs) to keep DVE/ACT/PE pipelined
"""
import sys

sys.path.insert(0, "/opt/trn_rl_repo")

import numpy as np
import ml_dtypes
from contextlib import ExitStack

import concourse.bass as bass
import concourse.tile as tile
import concourse.mybir as mybir
from concourse import bacc
from concourse.masks import make_identity

DT = mybir.dt
F32 = DT.float32
BF16 = DT.bfloat16
FP8 = DT.float8e4
AF = mybir.ActivationFunctionType
ALU = mybir.AluOpType

B, T, S = 32, 1000, 100
D, A, H, E, V, RO = 512, 1024, 1024, 640, 10240, 1024
ZH, ZC = 0.05, 0.15
NEG = -1e30
NCORE = 8
BL = B // NCORE          # 4 batches (slots) per core
NS = S * BL              # 400 step-batch columns
GC = 4 * H // 128        # 32 gate chunks
HC = H // 128            # 8
AC = A // 128            # 8
DC = D // 128            # 4
EC = E // 128            # 5
ROC = RO // 2 // 128     # 4 chunks per maxout half
VC = V // 128            # 80 vocab chunks
XROC = (H + E + D) // 128  # 17 readout K-chunks

bf16 = ml_dtypes.bfloat16
LAST_EXEC_NS = None
LAST_OUTS = None
LAST_META = None

E_T_DT = FP8  # dtype of the resident enc-context tiles


def _bf(a):
    return np.ascontiguousarray(np.asarray(a, dtype=np.float32)).astype(bf16)


def build_nc(Tp):
    """Build the SPMD graph. Tp: list of BL padded time extents (mult of 128)."""
    TC = [t // 128 for t in Tp]
    TCmax = max(TC)
    nc = bacc.Bacc("TRN2", target_bir_lowering=False)

    def param(name, shape, dt=BF16):
        return nc.declare_dram_parameter(name, list(shape), dt, isOutput=False)

    enc_td = [param(f"enc_td{j}", [Tp[j], D]) for j in range(BL)]
    encT = [param(f"encT{j}", [D, Tp[j]]) for j in range(BL)]
    embT_d = param("embT", [E, NS])
    W_combT_d = param("W_combT", [D + H, 4 * H])
    W_ih_embT_d = param("W_ih_embT", [E, 4 * H])
    W_encT_d = param("W_encT", [D, A])
    W_sT_d = param("W_sT", [H, A])
    wfert_col_d = param("wfert_col", [128, DC])
    vT_col_d = param("vT_col", [128, AC])
    # [128, (a, j, 4)]: column (a,j,j2) = v chunk a if j2 == j else 0 —
    # zero-padded so slot j's energy lands on PSUM partition j (PE matmul
    # output base partition must be 0/32/64, so we can't target row j
    # directly with a 1-column weight)
    vT4_d = param("vT4", [128, AC * BL * BL])
    wfb_col_d = param("wfb_col", [128, AC])
    b_enc_col_d = param("b_enc_col", [128, AC], F32)
    b_comb_d = param("b_comb", [128, GC], F32)
    m01_d = param("m01", [BL, 1024])
    W_roT_e_d = param("W_roT_e", [H + E + D, RO // 2])
    W_roT_o_d = param("W_roT_o", [H + E + D, RO // 2])
    b_ro_e_d = param("b_ro_e", [128, ROC], F32)
    b_ro_o_d = param("b_ro_o", [128, ROC], F32)
    W_outT_d = param("W_outT", [RO // 2, V])
    b_out_d = param("b_out_col", [128, VC], F32)
    out_d = nc.declare_dram_parameter("out", [V, NS], F32, isOutput=True)

    qd = nc.dram_tensor("qd", [BL, 1024], BF16)
    hstk_d = nc.dram_tensor("hstk", [H, NS], BF16)
    cstk_d = nc.dram_tensor("cstk", [D, NS], BF16)
    xembT_d = nc.dram_tensor("xembT", [4 * H, NS], BF16)

    with ExitStack() as ctx:
        tc = ctx.enter_context(tile.TileContext(nc))

        # ---------------- persistent pools ----------------
        persist = ctx.enter_context(tc.tile_pool(name="persist", bufs=1))
        ident = persist.tile([128, 128], BF16)
        make_identity(nc, ident[:])
        vT4 = persist.tile([128, AC * BL * BL], BF16)
        nc.sync.dma_start(vT4[:], vT4_d[:, :])
        wfb_col = persist.tile([128, AC], BF16)
        nc.sync.dma_start(wfb_col[:], wfb_col_d[:, :])
        wfert_col = persist.tile([128, DC], BF16)
        nc.sync.dma_start(wfert_col[:], wfert_col_d[:, :])
        b_enc_col = persist.tile([128, AC], F32)
        nc.sync.dma_start(b_enc_col[:], b_enc_col_d[:, :])
        b_comb = persist.tile([128, GC], F32)
        nc.sync.dma_start(b_comb[:], b_comb_d[:, :])
        m01 = persist.tile([BL, 1024], BF16)
        nc.sync.dma_start(m01[:], m01_d[:, :])
        b_ro_e = persist.tile([128, ROC], F32)
        nc.sync.dma_start(b_ro_e[:], b_ro_e_d[:, :])
        b_ro_o = persist.tile([128, ROC], F32)
        nc.sync.dma_start(b_ro_o[:], b_ro_o_d[:, :])
        b_out_col = persist.tile([128, VC], F32)
        nc.sync.dma_start(b_out_col[:], b_out_d[:, :])
        wzero = persist.tile([128, BL], BF16)
        nc.vector.memset(wzero[:], 0.0)

        h_bf = persist.tile([128, HC * BL], BF16)      # [128,(hchunk,b)]
        c_st = persist.tile([128, HC * BL], F32)
        ctxT_sb = persist.tile([128, DC * BL], BF16)   # [128,(dchunk,b)]
        s_sb = persist.tile([128, AC * BL], F32)       # [128,(achunk,b)]
        accum = persist.tile([BL, 1024], F32)
        accum_bf = persist.tile([BL, 1024], BF16)
        En_sb = persist.tile([BL, 1024], BF16)
        w_att_n = persist.tile([BL, 1024], BF16)
        finv05 = persist.tile([BL, 1024], BF16)
        gates_f = persist.tile([128, GC * BL], F32)    # [128,(gchunk,b)]
        q_bc = [persist.tile([128, Tp[j]], BF16, name=f"qbc{j}")
                for j in range(BL)]
        for t_ in (h_bf, c_st, ctxT_sb, s_sb, accum, accum_bf, En_sb,
                   w_att_n, finv05):
            nc.vector.memset(t_[:], 0.0)
        for j in range(BL):
            nc.vector.memset(q_bc[j][:], 0.0)

        inner = ctx.enter_context(ExitStack())
        e_pool = inner.enter_context(tc.tile_pool(name="e", bufs=1))
        enc_pool = inner.enter_context(tc.tile_pool(name="enc", bufs=1))
        e_t = [[e_pool.tile([128, Tp[j]], E_T_DT, name=f"e_{j}_{a}",
                            tag=f"e{j}{a}")
                for a in range(AC)] for j in range(BL)]
        # resident encoder tiles: [128t, 512d] per (slot, t-chunk)
        enc_sb = [[enc_pool.tile([128, D], BF16, name=f"es{j}_{t}",
                                 tag=f"es{j}{t}")
                   for t in range(TC[j])] for j in range(BL)]

        ppsum = ctx.enter_context(tc.tile_pool(name="ppsum", bufs=1, space="PSUM"))
        G = ppsum.tile([128, GC * BL], F32)
        Ghh = ppsum.tile([128, GC * BL], F32)
        trash_ps = ppsum.tile([128, 128], BF16)

        def pe_touch(ap):
            # absorb a foreign clock into PE via a transpose that reads ap
            p = ap.shape[0]
            nc.tensor.transpose(trash_ps[0:min(ap.shape[1], 128), 0:p],
                                ap[:, 0:min(ap.shape[1], 128)], ident[0:p, 0:p])

        # ============ PRECOMPUTE PHASE ============
        with ExitStack() as pre:
            pre_sb = pre.enter_context(tc.tile_pool(name="pre_sb", bufs=1))
            pre_st = pre.enter_context(tc.tile_pool(name="pre_st", bufs=2))
            pre_ps = pre.enter_context(tc.tile_pool(name="pre_ps", bufs=1,
                                                    space="PSUM"))

            # resident enc tiles (persist across the whole loop)
            for j in range(BL):
                for t in range(TC[j]):
                    nc.sync.dma_start(enc_sb[j][t][:],
                                      enc_td[j][t * 128:(t + 1) * 128, :])

            W_encT = [pre_sb.tile([128, A], BF16, name=f"wenc{k}", tag=f"we{k}")
                      for k in range(DC)]
            for k in range(DC):
                nc.sync.dma_start(W_encT[k][:], W_encT_d[k * 128:(k + 1) * 128, :])
            pe_touch(W_encT[0][:, 0:128])

            for j in range(BL):
                ercs = [pre_st.tile([128, Tp[j]], BF16, name=f"erc{j}{k}",
                                    tag=f"erc{k}") for k in range(DC)]
                for k in range(DC):
                    nc.sync.dma_start(ercs[k][:], encT[j][k * 128:(k + 1) * 128, :])
                    pe_touch(ercs[k][:, 0:128])
                for a in range(AC):
                    pe2 = pre_ps.tile([128, 1024], F32, name="pe_e2", tag="pe_e2")
                    for k in range(DC):
                        for n0 in range(0, Tp[j], 512):
                            n1 = min(n0 + 512, Tp[j])
                            nc.tensor.matmul(pe2[:, n0:n1],
                                             W_encT[k][:, a * 128:(a + 1) * 128],
                                             ercs[k][:, n0:n1],
                                             start=(k == 0), stop=(k == DC - 1))
                    nc.scalar.activation(e_t[j][a][:], pe2[:, 0:Tp[j]],
                                         AF.Identity,
                                         bias=b_enc_col[:, a:a + 1], scale=1.0)
                pf = pre_ps.tile([1, 1024], F32, name="pf", tag="pf")
                for k in range(DC):
                    for n0 in range(0, Tp[j], 512):
                        n1 = min(n0 + 512, Tp[j])
                        nc.tensor.matmul(pf[0:1, n0:n1], wfert_col[:, k:k + 1],
                                         ercs[k][:, n0:n1],
                                         start=(k == 0), stop=(k == DC - 1))
                fstage = pre_st.tile([1, 1024], BF16, name="fstage", tag="fstage")
                nc.scalar.activation(fstage[0:1, 0:Tp[j]], pf[0:1, 0:Tp[j]],
                                     AF.Sigmoid)
                nc.sync.dma_start(finv05[j:j + 1, 0:Tp[j]], fstage[0:1, 0:Tp[j]])
            nc.vector.tensor_scalar(finv05[:], finv05[:], 0.5, None, ALU.mult)

            embT_sb = [pre_sb.tile([128, NS], BF16, name=f"embs{k}", tag=f"em{k}")
                       for k in range(EC)]
            for k in range(EC):
                nc.sync.dma_start(embT_sb[k][:], embT_d[k * 128:(k + 1) * 128, :])
            W_ie = [pre_sb.tile([128, 4 * H], BF16, name=f"wie{k}", tag=f"wi{k}")
                    for k in range(EC)]
            for k in range(EC):
                nc.sync.dma_start(W_ie[k][:], W_ih_embT_d[k * 128:(k + 1) * 128, :])
            pe_touch(W_ie[0][:, 0:128])
            pe_touch(embT_sb[0][:, 0:128])
            for g in range(GC):
                px = pre_ps.tile([128, NS], F32, name="px", tag="pe_e2")
                for k in range(EC):
                    nc.tensor.matmul(px[:], W_ie[k][:, g * 128:(g + 1) * 128],
                                     embT_sb[k][:], start=(k == 0),
                                     stop=(k == EC - 1))
                stg = pre_st.tile([128, NS], BF16, name="xstg", tag="xstg")
                nc.scalar.activation(stg[:], px[:], AF.Identity,
                                     bias=b_comb[:, g:g + 1], scale=1.0)
                nc.sync.dma_start(xembT_d[g * 128:(g + 1) * 128, :], stg[:])

        # ============ WEIGHTS (loop phase) ============
        w_pool = inner.enter_context(tc.tile_pool(name="w", bufs=1))
        W_comb = [w_pool.tile([128, 4 * H], BF16, name=f"wc{k}", tag=f"wc{k}")
                  for k in range(DC + HC)]
        for k in range(DC + HC):
            nc.sync.dma_start(W_comb[k][:], W_combT_d[k * 128:(k + 1) * 128, :])
        W_sT = [w_pool.tile([128, A], BF16, name=f"ws{k}", tag=f"ws{k}")
                for k in range(HC)]
        for k in range(HC):
            nc.sync.dma_start(W_sT[k][:], W_sT_d[k * 128:(k + 1) * 128, :])

        loop_sb = inner.enter_context(tc.tile_pool(name="lsb", bufs=2))
        z_pool = inner.enter_context(tc.tile_pool(name="lz", bufs=3))
        tv_pool = inner.enter_context(tc.tile_pool(name="ltv", bufs=3))
        xet_pool = inner.enter_context(tc.tile_pool(name="lxe", bufs=2))
        loop_ps = inner.enter_context(tc.tile_pool(name="lps", bufs=1,
                                                   space="PSUM"))

        # prologue: W_hh part of step 0 with h=0 (initializes psum groups)
        pe_touch(W_comb[DC][:, 0:128])
        pe_touch(h_bf[:, 0:HC * BL])
        for g in range(GC):
            for k in range(HC):
                nc.tensor.matmul(Ghh[:, g * BL:(g + 1) * BL],
                                 W_comb[DC + k][:, g * 128:(g + 1) * 128],
                                 h_bf[:, k * BL:(k + 1) * BL],
                                 start=(k == 0), stop=(k == HC - 1),
                                 skip_group_check=True)

        # ============ STEP LOOP ============
        import os as _os
        _nsteps = int(_os.environ.get("KBSTEPS", S))
        with tc.For_i(0, _nsteps * BL, BL) as t4:
            # xemb tile for this step: [128, (gchunk, b)]
            xet = xet_pool.tile([128, GC * BL], BF16, name="xet", tag="xet")
            src = xembT_d[:, bass.ds(t4, BL)].rearrange("(c p) b -> p c b", p=128)
            nc.sync.dma_start(xet[:], src)

            # gates: ctx part (uses ctxT_sb of prev step), by gate quarter so
            # the LSTM activations can start before all 32 g-chunks finish
            QW = GC // 4 * BL   # columns per gate quarter (8 chunks * 4)
            tau = []            # tanh(i/2), tanh(f/2), tanh(g), tanh(o/2)
            for q in range(4):
                for g in range(q * GC // 4, (q + 1) * GC // 4):
                    for k in range(DC):
                        nc.tensor.matmul(G[:, g * BL:(g + 1) * BL],
                                         W_comb[k][:, g * 128:(g + 1) * 128],
                                         ctxT_sb[:, k * BL:(k + 1) * BL],
                                         start=(k == 0), stop=(k == DC - 1),
                                         skip_group_check=True)
                sl = slice(q * QW, (q + 1) * QW)
                # one PSUM input max per DVE op: G+xet first, then +Ghh
                nc.vector.tensor_tensor(gates_f[:, sl], G[:, sl], xet[:, sl],
                                        ALU.add)
                nc.vector.tensor_tensor(gates_f[:, sl], gates_f[:, sl],
                                        Ghh[:, sl], ALU.add)
                tq = loop_sb.tile([128, QW], F32, name=f"tau{q}", tag=f"tau{q}")
                nc.scalar.activation(tq[:], gates_f[:, sl], AF.Tanh,
                                     scale=(1.0 if q == 2 else 0.5))
                tau.append(tq)

            # all-tanh LSTM:  sig(x) = (1 + tanh(x/2)) / 2
            #   c_new = c*(0.5*tf + 0.5) + tg*(0.5*ti + 0.5)
            #   h'    = 0.05*h + tanh(c_new)*(0.475*to + 0.475)
            #   c'    = 0.15*c + 0.85*c_new
            ti, tf, tg, to = tau
            u = loop_sb.tile([128, QW], F32, name="u", tag="u")
            nc.vector.tensor_scalar(u[:], tf[:], 0.5, 0.5, ALU.mult, ALU.add)
            c1 = loop_sb.tile([128, QW], F32, name="c1", tag="c1")
            nc.vector.tensor_tensor(c1[:], u[:], c_st[:], ALU.mult)
            v = loop_sb.tile([128, QW], F32, name="v", tag="v")
            nc.vector.tensor_scalar(v[:], ti[:], 0.5, 0.5, ALU.mult, ALU.add)
            c2 = loop_sb.tile([128, QW], F32, name="c2", tag="c2")
            nc.vector.tensor_tensor(c2[:], v[:], tg[:], ALU.mult)
            cn = loop_sb.tile([128, QW], F32, name="cn", tag="cn")
            nc.vector.tensor_tensor(cn[:], c1[:], c2[:], ALU.add)
            tc_t = loop_sb.tile([128, QW], F32, name="tc_t", tag="tc_t")
            nc.scalar.activation(tc_t[:], cn[:], AF.Tanh)
            c3 = loop_sb.tile([128, QW], F32, name="c3", tag="c3")
            nc.vector.tensor_scalar(c3[:], c_st[:], ZC, None, ALU.mult)
            nc.vector.scalar_tensor_tensor(c_st[:], cn[:], 1.0 - ZC, c3[:],
                                           ALU.mult, ALU.add)
            w_ = loop_sb.tile([128, QW], F32, name="w_", tag="w_")
            nc.vector.tensor_scalar(w_[:], to[:], 0.475, 0.475, ALU.mult, ALU.add)
            h1 = loop_sb.tile([128, QW], F32, name="h1", tag="h1")
            nc.vector.tensor_tensor(h1[:], w_[:], tc_t[:], ALU.mult)
            h2 = loop_sb.tile([128, QW], F32, name="h2", tag="h2")
            nc.vector.tensor_scalar(h2[:], h_bf[:], ZH, None, ALU.mult)
            nc.vector.tensor_tensor(h_bf[:], h1[:], h2[:], ALU.add)

            # store h to DRAM stack
            dst = hstk_d[:, bass.ds(t4, BL)].rearrange("(c p) b -> p c b", p=128)
            nc.sync.dma_start(dst, h_bf[:])

            # s_t (a-major so the first attention chunks unblock early)
            s_ps = loop_ps.tile([128, AC * BL], F32, name="s_ps", tag="s_ps")
            for a in range(AC):
                for k in range(HC):
                    nc.tensor.matmul(s_ps[:, a * BL:(a + 1) * BL],
                                     W_sT[k][:, a * 128:(a + 1) * 128],
                                     h_bf[:, k * BL:(k + 1) * BL],
                                     start=(k == 0), stop=(k == HC - 1))
            nc.vector.tensor_copy(s_sb[:], s_ps[:])

            # ---- attention: z (DVE) -> tanh (ACT) -> energy MMs (PE) ----
            # Eps4[j, t] accumulates v . tanh(...) for slot j; one PSUM
            # accumulation group for all slots (start wipes each bank once
            # via slot 0 which always spans the full 1024 columns).
            Eps4 = loop_ps.tile([BL, 1024], F32, name="Eps4", tag="Eps4")
            for j in range(BL):
                Tpj = Tp[j]
                for a in range(AC):
                    z = z_pool.tile([128, 1024], BF16, name="z", tag="z")
                    nc.vector.scalar_tensor_tensor(
                        z[:, 0:Tpj], q_bc[j][:, 0:Tpj], wfb_col[:, a:a + 1],
                        e_t[j][a][:, 0:Tpj], ALU.mult, ALU.add)
                    tv = tv_pool.tile([128, 1024], BF16, name="tv", tag="tv")
                    nc.scalar.activation(tv[:, 0:Tpj], z[:, 0:Tpj], AF.Tanh,
                                         bias=s_sb[:, a * BL + j:a * BL + j + 1],
                                         scale=1.0)
                    w4 = vT4[:, (a * BL + j) * BL:(a * BL + j) * BL + BL]
                    for n0 in range(0, Tpj, 512):
                        n1 = min(n0 + 512, Tpj)
                        nc.tensor.matmul(Eps4[0:BL, n0:n1],
                                         w4, tv[:, n0:n1],
                                         start=(j == 0 and a == 0),
                                         stop=(j == BL - 1 and a == AC - 1),
                                         skip_group_check=True)
                    if j == 0 and a == 0 and min(Tp) < 1024:
                        # write zeros over the tail columns (all rows) once
                        # the banks' has_written bits have been wiped; later
                        # real MMs accumulate on top
                        nc.tensor.matmul(
                            Eps4[0:BL, min(Tp):1024], wzero[:, 0:BL],
                            W_sT[0][:, 0:1024 - min(Tp)],
                            start=False, stop=False, skip_group_check=True)

            # W_hh part of NEXT step's gates (after energies so the PE never
            # starves the attention pipeline)
            for g in range(GC):
                for k in range(HC):
                    nc.tensor.matmul(Ghh[:, g * BL:(g + 1) * BL],
                                     W_comb[DC + k][:, g * 128:(g + 1) * 128],
                                     h_bf[:, k * BL:(k + 1) * BL],
                                     start=(k == 0), stop=(k == HC - 1),
                                     skip_group_check=True)

            # ---- masked softmax straight from PSUM ----
            nmx = loop_sb.tile([BL, 1], F32, name="nmx", tag="nmx")
            nc.vector.tensor_reduce(nmx[:], Eps4[0:BL, :], mybir.AxisListType.X,
                                    ALU.max, negate=True)
            nc.scalar.activation(En_sb[:], Eps4[0:BL, :], AF.Exp, bias=nmx[:],
                                 scale=1.0)
            se = loop_sb.tile([BL, 1], F32, name="se", tag="se")
            nc.vector.scalar_tensor_tensor(En_sb[:], En_sb[:], 1.0, m01[:],
                                           ALU.mult, ALU.mult, accum_out=se[:])
            rse = loop_sb.tile([BL, 1], F32, name="rse", tag="rse")
            nc.vector.reciprocal(rse[:], se[:])
            nc.vector.tensor_scalar(w_att_n[:], En_sb[:], rse[:], None, ALU.mult)

            # accum += w_att_n * finv * 0.5 ; broadcast q for next step
            nc.vector.tensor_tensor(En_sb[:], w_att_n[:], finv05[:], ALU.mult)
            nc.vector.tensor_tensor(accum[:], accum[:], En_sb[:], ALU.add)
            nc.vector.tensor_copy(accum_bf[:], accum[:])
            nc.sync.dma_start(qd[:, :], accum_bf[:])
            for j in range(BL):
                nc.sync.dma_start(
                    q_bc[j][:], qd[j:j + 1, 0:Tp[j]].partition_broadcast(128))

            # transpose w_att_n chunks -> [128, (tc, b)]
            wtp = loop_ps.tile([128, TCmax * BL], BF16, name="wtp", tag="wtp")
            for t in range(TCmax):
                nc.tensor.transpose(wtp[:, t * BL:(t + 1) * BL],
                                    w_att_n[0:BL, t * 128:(t + 1) * 128],
                                    ident[0:BL, 0:BL])
            wts = loop_sb.tile([128, TCmax * BL], BF16, name="wts", tag="wts")
            nc.vector.tensor_copy(wts[:], wtp[:])

            # ctx: enc-stationary matvecs over the resident enc tiles
            c_ps = loop_ps.tile([128, DC * BL], F32, name="c_ps", tag="c_ps")
            for j in range(BL):
                for dk in range(DC):
                    for t in range(TC[j]):
                        nc.tensor.matmul(
                            c_ps[:, dk * BL + j:dk * BL + j + 1],
                            enc_sb[j][t][:, dk * 128:(dk + 1) * 128],
                            wts[:, t * BL + j:t * BL + j + 1],
                            start=(t == 0), stop=(t == TC[j] - 1))
            nc.vector.tensor_copy(ctxT_sb[:], c_ps[:])
            dst = cstk_d[:, bass.ds(t4, BL)].rearrange("(c p) b -> p c b", p=128)
            nc.sync.dma_start(dst, ctxT_sb[:])

        # ============ READOUT ============
        inner.close()
        post_sb = ctx.enter_context(tc.tile_pool(name="post_sb", bufs=1))
        post_st = ctx.enter_context(tc.tile_pool(name="post_st", bufs=2))
        post_ps = ctx.enter_context(tc.tile_pool(name="post_ps", bufs=2,
                                                 space="PSUM"))

        # stage xro moving tiles: [s(8) | emb(5) | ctx(4)] chunks of [128, NS]
        xro = []
        for k in range(HC):
            tl = post_sb.tile([128, NS], BF16, name=f"xh{k}", tag=f"xh{k}")
            nc.sync.dma_start(tl[:], hstk_d[k * 128:(k + 1) * 128, :])
            xro.append(tl)
        for k in range(EC):
            tl = post_sb.tile([128, NS], BF16, name=f"xe{k}", tag=f"xe{k}")
            nc.sync.dma_start(tl[:], embT_d[k * 128:(k + 1) * 128, :])
            xro.append(tl)
        for k in range(DC):
            tl = post_sb.tile([128, NS], BF16, name=f"xc{k}", tag=f"xc{k}")
            nc.sync.dma_start(tl[:], cstk_d[k * 128:(k + 1) * 128, :])
            xro.append(tl)
        W_roe = [post_sb.tile([128, RO // 2], BF16, name=f"wre{k}", tag=f"wre{k}")
                 for k in range(XROC)]
        W_roo = [post_sb.tile([128, RO // 2], BF16, name=f"wro{k}", tag=f"wro{k}")
                 for k in range(XROC)]
        for k in range(XROC):
            nc.sync.dma_start(W_roe[k][:], W_roT_e_d[k * 128:(k + 1) * 128, :])
            nc.sync.dma_start(W_roo[k][:], W_roT_o_d[k * 128:(k + 1) * 128, :])
        pe_touch(xro[0][:, 0:128])
        pe_touch(W_roe[0][:, 0:128])
        pe_touch(W_roo[0][:, 0:128])

        maxo = []
        for oc in range(ROC):
            Re = post_ps.tile([128, NS], F32, name="Re", tag="Re")
            for k in range(XROC):
                nc.tensor.matmul(Re[:], W_roe[k][:, oc * 128:(oc + 1) * 128],
                                 xro[k][:], start=(k == 0), stop=(k == XROC - 1))
            t1 = post_st.tile([128, NS], F32, name="t1", tag="t1")
            nc.scalar.activation(t1[:], Re[:], AF.Identity,
                                 bias=b_ro_e[:, oc:oc + 1], scale=1.0)
            Ro = post_ps.tile([128, NS], F32, name="Ro", tag="Re")
            for k in range(XROC):
                nc.tensor.matmul(Ro[:], W_roo[k][:, oc * 128:(oc + 1) * 128],
                                 xro[k][:], start=(k == 0), stop=(k == XROC - 1))
            t2 = post_st.tile([128, NS], F32, name="t2", tag="t2")
            nc.scalar.activation(t2[:], Ro[:], AF.Identity,
                                 bias=b_ro_o[:, oc:oc + 1], scale=1.0)
            mo = post_sb.tile([128, NS], BF16, name=f"mo{oc}", tag=f"mo{oc}")
            nc.vector.tensor_tensor(mo[:], t1[:], t2[:], ALU.max)
            maxo.append(mo)

        # logits
        wo_pool = ctx.enter_context(tc.tile_pool(name="wo", bufs=6))
        first = True
        for vc in range(VC):
            wo = [wo_pool.tile([128, 128], BF16, name=f"wo{vc}_{k}", tag=f"wok{k}")
                  for k in range(ROC)]
            for k in range(ROC):
                nc.sync.dma_start(wo[k][:],
                                  W_outT_d[k * 128:(k + 1) * 128,
                                           vc * 128:(vc + 1) * 128])
            if first:
                pe_touch(wo[0][:, 0:128])
                pe_touch(maxo[0][:, 0:128])
                first = False
            L = post_ps.tile([128, NS], F32, name="L", tag="L")
            for k in range(ROC):
                nc.tensor.matmul(L[:], wo[k][:], maxo[k][:],
                                 start=(k == 0), stop=(k == ROC - 1))
            lo = post_st.tile([128, NS], F32, name="lo", tag="lo")
            nc.scalar.activation(lo[:], L[:], AF.Identity,
                                 bias=b_out_col[:, vc:vc + 1], scale=1.0)
            nc.sync.dma_start(out_d[vc * 128:(vc + 1) * 128, :], lo[:])

    return nc


def check_waits(nc, cap_note=""):
    """Print compute instructions with >=2 sync waits (walrus limit is 1)."""
    bad = []
    for fn in nc.m.functions:
        for bb in fn.blocks:
            for inst in bb.instructions:
                c = inst.concise()
                nw = c.count("wait:")
                eng = c.split()[0] if c.split() else "?"
                if nw >= 2 and eng in ("PE", "ACT", "DVE", "PL"):
                    bad.append((nw, c[:180]))
    for nw, c in bad:
        print("WAITS", nw, c)
    return bad


def _make_vT4(v_att):
    """[128, (a, j, 4)]: col (a,j,j2) = v chunk a if j2 == j else 0."""
    v = v_att.reshape(AC, 128)  # [a, 128]
    out = np.zeros((128, AC, BL, BL), np.float32)
    for a in range(AC):
        for j in range(BL):
            out[:, a, j, j] = v[a]
    return out.reshape(128, AC * BL * BL)


def _prep_core(inputs, order, Tp, core):
    """Host-side data prep for one core. Returns the in_map dict."""
    enc = np.asarray(inputs["encoder_outputs"], np.float32)
    labels = np.asarray(inputs["labels"])
    lens = np.asarray(inputs["enc_seq_len"], np.int64)
    embed = np.asarray(inputs["embed"], np.float32)

    bidx = [int(order[j * NCORE + core]) for j in range(BL)]
    m = {}
    for j in range(BL):
        b = bidx[j]
        ep = np.zeros((Tp[j], D), np.float32)
        ep[:T] = enc[b, :Tp[j] if Tp[j] <= T else T]
        m[f"enc_td{j}"] = _bf(ep)
        m[f"encT{j}"] = _bf(ep.T)
    # shifted embeddings: [S, BL, E] -> embT [E, (s, j)]
    emb = np.zeros((BL, S, E), np.float32)
    for j in range(BL):
        b = bidx[j]
        emb[j, 1:] = embed[labels[b, :S - 1].astype(np.int64)]
    embT = emb.transpose(2, 1, 0).reshape(E, NS)  # (E, (s, j))
    m["embT"] = _bf(embT)
    # multiplicative mask (1 inside the sequence, 0 outside)
    m01 = np.zeros((BL, 1024), np.float32)
    for j in range(BL):
        m01[j, :int(lens[bidx[j]])] = 1.0
    m["m01"] = _bf(m01)
    return m, bidx


def kernel(**inputs):
    lens = np.asarray(inputs["enc_seq_len"], np.int64)
    order = np.argsort(-lens, kind="stable")
    Tp = []
    for j in range(BL):
        mx = max(int(lens[order[j * NCORE + i]]) for i in range(NCORE))
        Tp.append(min(1024, ((mx + 127) // 128) * 128))

    W_ih = np.asarray(inputs["W_ih"], np.float32)
    W_hh = np.asarray(inputs["W_hh"], np.float32)
    shared = {
        "W_combT": _bf(np.concatenate([W_ih[:, E:].T, W_hh.T], 0)),
        "W_ih_embT": _bf(W_ih[:, :E].T),
        "W_encT": _bf(np.asarray(inputs["W_enc"], np.float32).T),
        "W_sT": _bf(np.asarray(inputs["W_s"], np.float32).T),
        "wfert_col": _bf(np.asarray(inputs["W_fert"],
                                    np.float32).reshape(DC, 128).T),
        "vT_col": _bf(np.asarray(inputs["v_att"], np.float32).reshape(AC, 128).T),
        "vT4": _bf(_make_vT4(np.asarray(inputs["v_att"], np.float32))),
        "wfb_col": _bf(np.asarray(inputs["W_fb"], np.float32)[:, 0]
                       .reshape(AC, 128).T),
        "b_enc_col": np.ascontiguousarray(
            np.asarray(inputs["b_enc"], np.float32).reshape(AC, 128).T),
        "b_comb": np.ascontiguousarray(
            (np.asarray(inputs["b_ih"], np.float32)
             + np.asarray(inputs["b_hh"], np.float32)).reshape(GC, 128).T),
        "W_roT_e": _bf(np.asarray(inputs["W_ro"], np.float32)[0::2].T),
        "W_roT_o": _bf(np.asarray(inputs["W_ro"], np.float32)[1::2].T),
        "b_ro_e": np.ascontiguousarray(
            np.asarray(inputs["b_ro"], np.float32)[0::2].reshape(ROC, 128).T),
        "b_ro_o": np.ascontiguousarray(
            np.asarray(inputs["b_ro"], np.float32)[1::2].reshape(ROC, 128).T),
        "W_outT": _bf(np.asarray(inputs["W_out"], np.float32).T),
        "b_out_col": np.ascontiguousarray(
            np.asarray(inputs["b_out"], np.float32).reshape(VC, 128).T),
    }

    in_maps = []
    bidx_all = []
    for c in range(NCORE):
        m, bidx = _prep_core(inputs, order, Tp, c)
        m.update(shared)
        in_maps.append(m)
        bidx_all.append(bidx)

    nc = build_nc(Tp)
    nc.finalize()
    from concourse.bass_utils import run_bass_kernel_spmd
    import os as _os
    trace = bool(_os.environ.get("BASS_KERNEL_TRACE"))
    res = run_bass_kernel_spmd(nc, in_maps, core_ids=list(range(NCORE)),
                               trace=trace)
    global LAST_EXEC_NS, LAST_OUTS, LAST_META
    LAST_EXEC_NS = res.exec_time_ns
    outs = res.results
    LAST_OUTS = outs
    LAST_META = (order, Tp, bidx_all)

    logits = np.zeros((B, S, V), np.float32)
    for c in range(NCORE):
        o = outs[c]["out"].reshape(V, S, BL)
        for j in range(BL):
            logits[bidx_all[c][j]] = o[:, :, j].T
    return logits


if __name__ == "__main__":
    # quick build + wait check
    nc = build_nc([1024, 896, 768, 640])
    bad = check_waits(nc)
    print(f"{len(bad)} instructions with >=2 waits")


# revision 14
# speedup vs baseline: 1.0589x; 1.0589x over previous
"""Attention-LSTM decoder (B=32, T=1000, S=100, D=512, A=1024, H=1024,
E=640, V=10240, P=1024) on 8 trn2 NeuronCores.

Sharding: data-parallel over batch, 4 batches per core (one per "slot").
Batches are sorted by enc_seq_len; slot j holds ranks [j*8:(j+1)*8] so the
padded time extent Tp[j] (multiple of 128) is shared by all 8 cores and the
SPMD graph is identical across cores while skipping work beyond each slot's
padded length.

v2 restructure vs baseline:
  - encoder tiles resident in SBUF (no per-step enc DMA reload)
  - e_t (precomputed enc context) stored fp8e4m3 to make SBUF room
  - all-tanh LSTM (sigmoid(x) = (1+tanh(x/2))/2 folded into the zoneout
    constants) so tanh+exp share one ACT table set -> zero per-step
    ACT_TABLE_LOADs
  - energies accumulate straight into a [4,1024] PSUM tile (out-partition
    = slot) -> softmax reads PSUM; no per-slot staging copies/DMAs
  - multiplicative 0/1 mask fused into the post-exp normalization
  - program order: gates -> LSTM -> s_t -> attention (z/tanh/energy MMs)
    -> W_hh for next step -> softmax -> q broadcast -> transposes -> ctx,
    so the PE never blocks the z/tanh pipeline and DVE z-ops overlap the
    gates phase of the next step
  - deep z/tv pools (4 bufs) to keep DVE/ACT/PE pipelined
"""
import sys

sys.path.insert(0, "/opt/trn_rl_repo")

import numpy as np
import ml_dtypes
from contextlib import ExitStack

import concourse.bass as bass
import concourse.tile as tile
import concourse.mybir as mybir
from concourse import bacc
from concourse.masks import make_identity

DT = mybir.dt
F32 = DT.float32
BF16 = DT.bfloat16
FP8 = DT.float8e4
AF = mybir.ActivationFunctionType
ALU = mybir.AluOpType

B, T, S = 32, 1000, 100
D, A, H, E, V, RO = 512, 1024, 1024, 640, 10240, 1024
ZH, ZC = 0.05, 0.15
NEG = -1e30
NCORE = 8
BL = B // NCORE          # 4 batches (slots) per core
NS = S * BL              # 400 step-batch columns
GC = 4 * H // 128        # 32 gate chunks
HC = H // 128            # 8
AC = A // 128            # 8
DC = D // 128            # 4
EC = E // 128            # 5
ROC = RO // 2 // 128     # 4 chunks per maxout half
VC = V // 128            # 80 vocab chunks
XROC = (H + E + D) // 128  # 17 readout K-chunks

bf16 = ml_dtypes.bfloat16
LAST_EXEC_NS = None
LAST_OUTS = None
LAST_META = None

E_T_DT = FP8  # dtype of the resident enc-context tiles


def _bf(a):
    return np.ascontiguousarray(np.asarray(a, dtype=np.float32)).astype(bf16)


def build_nc(Tp):
    """Build the SPMD graph. Tp: list of BL padded time extents (mult of 128)."""
    TC = [t // 128 for t in Tp]
    TCmax = max(TC)
    nc = bacc.Bacc("TRN2", target_bir_lowering=False)

    def param(name, shape, dt=BF16):
        return nc.declare_dram_parameter(name, list(shape), dt, isOutput=False)

    enc_td = [param(f"enc_td{j}", [Tp[j], D]) for j in range(BL)]
    encT = [param(f"encT{j}", [D, Tp[j]]) for j in range(BL)]
    embT_d = param("embT", [E, NS])
    W_combT_d = param("W_combT", [D + H, 4 * H])
    W_ih_embT_d = param("W_ih_embT", [E, 4 * H])
    W_encT_d = param("W_encT", [D, A])
    W_sT_d = param("W_sT", [H, A])
    wfert_col_d = param("wfert_col", [128, DC])
    vT_col_d = param("vT_col", [128, AC])
    # [128, (a, j, 4)]: column (a,j,j2) = v chunk a if j2 == j else 0 —
    # zero-padded so slot j's energy lands on PSUM partition j (PE matmul
    # output base partition must be 0/32/64, so we can't target row j
    # directly with a 1-column weight)
    vT4_d = param("vT4", [128, AC * BL * BL])
    wfb_col_d = param("wfb_col", [128, AC])
    b_enc_col_d = param("b_enc_col", [128, AC], F32)
    b_comb_d = param("b_comb", [128, GC], F32)
    m01_d = param("m01", [BL, 1024])
    W_roT_e_d = param("W_roT_e", [H + E + D, RO // 2])
    W_roT_o_d = param("W_roT_o", [H + E + D, RO // 2])
    b_ro_e_d = param("b_ro_e", [128, ROC], F32)
    b_ro_o_d = param("b_ro_o", [128, ROC], F32)
    W_outT_d = param("W_outT", [RO // 2, V])
    b_out_d = param("b_out_col", [128, VC], F32)
    out_d = nc.declare_dram_parameter("out", [V, NS], F32, isOutput=True)

    qd = nc.dram_tensor("qd", [BL, 1024], BF16)
    hstk_d = nc.dram_tensor("hstk", [H, NS], BF16)
    cstk_d = nc.dram_tensor("cstk", [D, NS], BF16)
    xembT_d = nc.dram_tensor("xembT", [4 * H, NS], BF16)

    with ExitStack() as ctx:
        tc = ctx.enter_context(tile.TileContext(nc))

        # ---------------- persistent pools ----------------
        persist = ctx.enter_context(tc.tile_pool(name="persist", bufs=1))
        ident = persist.tile([128, 128], BF16)
        make_identity(nc, ident[:])
        vT4 = persist.tile([128, AC * BL * BL], BF16)
        nc.sync.dma_start(vT4[:], vT4_d[:, :])
        wfb_col = persist.tile([128, AC], BF16)
        nc.sync.dma_start(wfb_col[:], wfb_col_d[:, :])
        wfert_col = persist.tile([128, DC], BF16)
        nc.sync.dma_start(wfert_col[:], wfert_col_d[:, :])
        b_enc_col = persist.tile([128, AC], F32)
        nc.sync.dma_start(b_enc_col[:], b_enc_col_d[:, :])
        b_comb = persist.tile([128, GC], F32)
        nc.sync.dma_start(b_comb[:], b_comb_d[:, :])
        m01 = persist.tile([BL, 1024], BF16)
        nc.sync.dma_start(m01[:], m01_d[:, :])
        b_ro_e = persist.tile([128, ROC], F32)
        nc.sync.dma_start(b_ro_e[:], b_ro_e_d[:, :])
        b_ro_o = persist.tile([128, ROC], F32)
        nc.sync.dma_start(b_ro_o[:], b_ro_o_d[:, :])
        b_out_col = persist.tile([128, VC], F32)
        nc.sync.dma_start(b_out_col[:], b_out_d[:, :])
        wzero = persist.tile([128, BL], BF16)
        nc.vector.memset(wzero[:], 0.0)

        h_bf = persist.tile([128, HC * BL], BF16)      # [128,(hchunk,b)]
        c_st = persist.tile([128, HC * BL], F32)
        ctxT_sb = persist.tile([128, DC * BL], BF16)   # [128,(dchunk,b)]
        s_sb = persist.tile([128, AC * BL], F32)       # [128,(achunk,b)]
        accum = persist.tile([BL, 1024], F32)
        accum_bf = persist.tile([BL, 1024], BF16)
        En_sb = persist.tile([BL, 1024], BF16)
        w_att_n = persist.tile([BL, 1024], BF16)
        finv05 = persist.tile([BL, 1024], BF16)
        gates_f = persist.tile([128, GC * BL], F32)    # [128,(gchunk,b)]
        q_bc = [persist.tile([128, Tp[j]], BF16, name=f"qbc{j}")
                for j in range(BL)]
        for t_ in (h_bf, c_st, ctxT_sb, s_sb, accum, accum_bf, En_sb,
                   w_att_n, finv05):
            nc.vector.memset(t_[:], 0.0)
        for j in range(BL):
            nc.vector.memset(q_bc[j][:], 0.0)

        inner = ctx.enter_context(ExitStack())
        e_pool = inner.enter_context(tc.tile_pool(name="e", bufs=1))
        enc_pool = inner.enter_context(tc.tile_pool(name="enc", bufs=1))
        e_t = [[e_pool.tile([128, Tp[j]], E_T_DT, name=f"e_{j}_{a}",
                            tag=f"e{j}{a}")
                for a in range(AC)] for j in range(BL)]
        # resident encoder tiles: [128t, 512d] per (slot, t-chunk)
        enc_sb = [[enc_pool.tile([128, D], BF16, name=f"es{j}_{t}",
                                 tag=f"es{j}{t}")
                   for t in range(TC[j])] for j in range(BL)]

        ppsum = ctx.enter_context(tc.tile_pool(name="ppsum", bufs=1, space="PSUM"))
        G = ppsum.tile([128, GC * BL], F32)
        Ghh = ppsum.tile([128, GC * BL], F32)
        trash_ps = ppsum.tile([128, 128], BF16)

        def pe_touch(ap):
            # absorb a foreign clock into PE via a transpose that reads ap
            p = ap.shape[0]
            nc.tensor.transpose(trash_ps[0:min(ap.shape[1], 128), 0:p],
                                ap[:, 0:min(ap.shape[1], 128)], ident[0:p, 0:p])

        # ============ PRECOMPUTE PHASE ============
        with ExitStack() as pre:
            pre_sb = pre.enter_context(tc.tile_pool(name="pre_sb", bufs=1))
            pre_st = pre.enter_context(tc.tile_pool(name="pre_st", bufs=2))
            pre_ps = pre.enter_context(tc.tile_pool(name="pre_ps", bufs=1,
                                                    space="PSUM"))

            # resident enc tiles (persist across the whole loop)
            for j in range(BL):
                for t in range(TC[j]):
                    nc.sync.dma_start(enc_sb[j][t][:],
                                      enc_td[j][t * 128:(t + 1) * 128, :])

            W_encT = [pre_sb.tile([128, A], BF16, name=f"wenc{k}", tag=f"we{k}")
                      for k in range(DC)]
            for k in range(DC):
                nc.sync.dma_start(W_encT[k][:], W_encT_d[k * 128:(k + 1) * 128, :])
            pe_touch(W_encT[0][:, 0:128])

            for j in range(BL):
                ercs = [pre_st.tile([128, Tp[j]], BF16, name=f"erc{j}{k}",
                                    tag=f"erc{k}") for k in range(DC)]
                for k in range(DC):
                    nc.sync.dma_start(ercs[k][:], encT[j][k * 128:(k + 1) * 128, :])
                    pe_touch(ercs[k][:, 0:128])
                for a in range(AC):
                    pe2 = pre_ps.tile([128, 1024], F32, name="pe_e2", tag="pe_e2")
                    for k in range(DC):
                        for n0 in range(0, Tp[j], 512):
                            n1 = min(n0 + 512, Tp[j])
                            nc.tensor.matmul(pe2[:, n0:n1],
                                             W_encT[k][:, a * 128:(a + 1) * 128],
                                             ercs[k][:, n0:n1],
                                             start=(k == 0), stop=(k == DC - 1))
                    nc.scalar.activation(e_t[j][a][:], pe2[:, 0:Tp[j]],
                                         AF.Identity,
                                         bias=b_enc_col[:, a:a + 1], scale=1.0)
                pf = pre_ps.tile([1, 1024], F32, name="pf", tag="pf")
                for k in range(DC):
                    for n0 in range(0, Tp[j], 512):
                        n1 = min(n0 + 512, Tp[j])
                        nc.tensor.matmul(pf[0:1, n0:n1], wfert_col[:, k:k + 1],
                                         ercs[k][:, n0:n1],
                                         start=(k == 0), stop=(k == DC - 1))
                fstage = pre_st.tile([1, 1024], BF16, name="fstage", tag="fstage")
                nc.scalar.activation(fstage[0:1, 0:Tp[j]], pf[0:1, 0:Tp[j]],
                                     AF.Sigmoid)
                nc.sync.dma_start(finv05[j:j + 1, 0:Tp[j]], fstage[0:1, 0:Tp[j]])
            nc.vector.tensor_scalar(finv05[:], finv05[:], 0.5, None, ALU.mult)

            embT_sb = [pre_sb.tile([128, NS], BF16, name=f"embs{k}", tag=f"em{k}")
                       for k in range(EC)]
            for k in range(EC):
                nc.sync.dma_start(embT_sb[k][:], embT_d[k * 128:(k + 1) * 128, :])
            W_ie = [pre_sb.tile([128, 4 * H], BF16, name=f"wie{k}", tag=f"wi{k}")
                    for k in range(EC)]
            for k in range(EC):
                nc.sync.dma_start(W_ie[k][:], W_ih_embT_d[k * 128:(k + 1) * 128, :])
            pe_touch(W_ie[0][:, 0:128])
            pe_touch(embT_sb[0][:, 0:128])
            for g in range(GC):
                px = pre_ps.tile([128, NS], F32, name="px", tag="pe_e2")
                for k in range(EC):
                    nc.tensor.matmul(px[:], W_ie[k][:, g * 128:(g + 1) * 128],
                                     embT_sb[k][:], start=(k == 0),
                                     stop=(k == EC - 1))
                stg = pre_st.tile([128, NS], BF16, name="xstg", tag="xstg")
                nc.scalar.activation(stg[:], px[:], AF.Identity,
                                     bias=b_comb[:, g:g + 1], scale=1.0)
                nc.sync.dma_start(xembT_d[g * 128:(g + 1) * 128, :], stg[:])

        # ============ WEIGHTS (loop phase) ============
        w_pool = inner.enter_context(tc.tile_pool(name="w", bufs=1))
        W_comb = [w_pool.tile([128, 4 * H], BF16, name=f"wc{k}", tag=f"wc{k}")
                  for k in range(DC + HC)]
        for k in range(DC + HC):
            nc.sync.dma_start(W_comb[k][:], W_combT_d[k * 128:(k + 1) * 128, :])
        W_sT = [w_pool.tile([128, A], BF16, name=f"ws{k}", tag=f"ws{k}")
                for k in range(HC)]
        for k in range(HC):
            nc.sync.dma_start(W_sT[k][:], W_sT_d[k * 128:(k + 1) * 128, :])

        loop_sb = inner.enter_context(tc.tile_pool(name="lsb", bufs=2))
        z_pool = inner.enter_context(tc.tile_pool(name="lz", bufs=3))
        tv_pool = inner.enter_context(tc.tile_pool(name="ltv", bufs=3))
        xet_pool = inner.enter_context(tc.tile_pool(name="lxe", bufs=2))
        loop_ps = inner.enter_context(tc.tile_pool(name="lps", bufs=1,
                                                   space="PSUM"))

        # prologue: W_hh part of step 0 with h=0 (initializes psum groups)
        pe_touch(W_comb[DC][:, 0:128])
        pe_touch(h_bf[:, 0:HC * BL])
        for g in range(GC):
            for k in range(HC):
                nc.tensor.matmul(Ghh[:, g * BL:(g + 1) * BL],
                                 W_comb[DC + k][:, g * 128:(g + 1) * 128],
                                 h_bf[:, k * BL:(k + 1) * BL],
                                 start=(k == 0), stop=(k == HC - 1),
                                 skip_group_check=True)

        # ============ STEP LOOP ============
        import os as _os
        _nsteps = int(_os.environ.get("KBSTEPS", S))
        with tc.For_i(0, _nsteps * BL, BL, staggered_reset=True,
                      hint_engines=tuple(mybir.ALL_ENGINES)) as t4:
            # q broadcast for THIS step's attention reads the accumulator
            # updated at the END of the previous iteration (loop-carried);
            # issuing it first hides the DMA latency behind the gates phase
            nc.sync.dma_start(qd[:, :], accum_bf[:])
            for j in range(BL):
                nc.sync.dma_start(
                    q_bc[j][:], qd[j:j + 1, 0:Tp[j]].partition_broadcast(128))

            # xemb tile for this step: [128, (gchunk, b)]
            xet = xet_pool.tile([128, GC * BL], BF16, name="xet", tag="xet")
            src = xembT_d[:, bass.ds(t4, BL)].rearrange("(c p) b -> p c b", p=128)
            nc.sync.dma_start(xet[:], src)

            # gates: ctx part (uses ctxT_sb of prev step), by gate quarter so
            # the LSTM activations can start before all 32 g-chunks finish
            QW = GC // 4 * BL   # columns per gate quarter (8 chunks * 4)
            tau = []            # tanh(i/2), tanh(f/2), tanh(g), tanh(o/2)
            for q in range(4):
                for g in range(q * GC // 4, (q + 1) * GC // 4):
                    for k in range(DC):
                        nc.tensor.matmul(G[:, g * BL:(g + 1) * BL],
                                         W_comb[k][:, g * 128:(g + 1) * 128],
                                         ctxT_sb[:, k * BL:(k + 1) * BL],
                                         start=(k == 0), stop=(k == DC - 1),
                                         skip_group_check=True)
                sl = slice(q * QW, (q + 1) * QW)
                # one PSUM input max per DVE op: G+xet first, then +Ghh
                nc.vector.tensor_tensor(gates_f[:, sl], G[:, sl], xet[:, sl],
                                        ALU.add)
                nc.vector.tensor_tensor(gates_f[:, sl], gates_f[:, sl],
                                        Ghh[:, sl], ALU.add)
                tq = loop_sb.tile([128, QW], F32, name=f"tau{q}", tag=f"tau{q}")
                nc.scalar.activation(tq[:], gates_f[:, sl], AF.Tanh,
                                     scale=(1.0 if q == 2 else 0.5))
                tau.append(tq)

            # all-tanh LSTM:  sig(x) = (1 + tanh(x/2)) / 2
            #   c_new = c*(0.5*tf + 0.5) + tg*(0.5*ti + 0.5)
            #   h'    = 0.05*h + tanh(c_new)*(0.475*to + 0.475)
            #   c'    = 0.15*c + 0.85*c_new
            ti, tf, tg, to = tau
            u = loop_sb.tile([128, QW], F32, name="u", tag="u")
            nc.vector.tensor_scalar(u[:], tf[:], 0.5, 0.5, ALU.mult, ALU.add)
            c1 = loop_sb.tile([128, QW], F32, name="c1", tag="c1")
            nc.vector.tensor_tensor(c1[:], u[:], c_st[:], ALU.mult)
            v = loop_sb.tile([128, QW], F32, name="v", tag="v")
            nc.vector.tensor_scalar(v[:], ti[:], 0.5, 0.5, ALU.mult, ALU.add)
            c2 = loop_sb.tile([128, QW], F32, name="c2", tag="c2")
            nc.vector.tensor_tensor(c2[:], v[:], tg[:], ALU.mult)
            cn = loop_sb.tile([128, QW], F32, name="cn", tag="cn")
            nc.vector.tensor_tensor(cn[:], c1[:], c2[:], ALU.add)
            tc_t = loop_sb.tile([128, QW], F32, name="tc_t", tag="tc_t")
            nc.scalar.activation(tc_t[:], cn[:], AF.Tanh)
            c3 = loop_sb.tile([128, QW], F32, name="c3", tag="c3")
            nc.vector.tensor_scalar(c3[:], c_st[:], ZC, None, ALU.mult)
            nc.vector.scalar_tensor_tensor(c_st[:], cn[:], 1.0 - ZC, c3[:],
                                           ALU.mult, ALU.add)
            w_ = loop_sb.tile([128, QW], F32, name="w_", tag="w_")
            nc.vector.tensor_scalar(w_[:], to[:], 0.475, 0.475, ALU.mult, ALU.add)
            h1 = loop_sb.tile([128, QW], F32, name="h1", tag="h1")
            nc.vector.tensor_tensor(h1[:], w_[:], tc_t[:], ALU.mult)
            h2 = loop_sb.tile([128, QW], F32, name="h2", tag="h2")
            nc.vector.tensor_scalar(h2[:], h_bf[:], ZH, None, ALU.mult)
            nc.vector.tensor_tensor(h_bf[:], h1[:], h2[:], ALU.add)

            # store h to DRAM stack
            dst = hstk_d[:, bass.ds(t4, BL)].rearrange("(c p) b -> p c b", p=128)
            nc.sync.dma_start(dst, h_bf[:])

            # s_t (a-major so the first attention chunks unblock early)
            s_ps = loop_ps.tile([128, AC * BL], F32, name="s_ps", tag="s_ps")
            for a in range(AC):
                for k in range(HC):
                    nc.tensor.matmul(s_ps[:, a * BL:(a + 1) * BL],
                                     W_sT[k][:, a * 128:(a + 1) * 128],
                                     h_bf[:, k * BL:(k + 1) * BL],
                                     start=(k == 0), stop=(k == HC - 1))
            nc.vector.tensor_copy(s_sb[:], s_ps[:])

            # ---- attention: z (DVE) -> tanh (ACT) -> energy MMs (PE) ----
            # Eps4[j, t] accumulates v . tanh(...) for slot j; one PSUM
            # accumulation group for all slots (start wipes each bank once
            # via slot 0 which always spans the full 1024 columns).
            Eps4 = loop_ps.tile([BL, 1024], F32, name="Eps4", tag="Eps4")
            for j in range(BL):
                Tpj = Tp[j]
                for a in range(AC):
                    z = z_pool.tile([128, 1024], BF16, name="z", tag="z")
                    nc.vector.scalar_tensor_tensor(
                        z[:, 0:Tpj], q_bc[j][:, 0:Tpj], wfb_col[:, a:a + 1],
                        e_t[j][a][:, 0:Tpj], ALU.mult, ALU.add)
                    tv = tv_pool.tile([128, 1024], BF16, name="tv", tag="tv")
                    nc.scalar.activation(tv[:, 0:Tpj], z[:, 0:Tpj], AF.Tanh,
                                         bias=s_sb[:, a * BL + j:a * BL + j + 1],
                                         scale=1.0)
                    w4 = vT4[:, (a * BL + j) * BL:(a * BL + j) * BL + BL]
                    for n0 in range(0, Tpj, 512):
                        n1 = min(n0 + 512, Tpj)
                        nc.tensor.matmul(Eps4[0:BL, n0:n1],
                                         w4, tv[:, n0:n1],
                                         start=(j == 0 and a == 0),
                                         stop=(j == BL - 1 and a == AC - 1),
                                         skip_group_check=True)
                    if j == 0 and a == 0 and min(Tp) < 1024:
                        # write zeros over the tail columns (all rows) once
                        # the banks' has_written bits have been wiped; later
                        # real MMs accumulate on top
                        nc.tensor.matmul(
                            Eps4[0:BL, min(Tp):1024], wzero[:, 0:BL],
                            W_sT[0][:, 0:1024 - min(Tp)],
                            start=False, stop=False, skip_group_check=True)

            # W_hh part of NEXT step's gates (after energies so the PE never
            # starves the attention pipeline)
            for g in range(GC):
                for k in range(HC):
                    nc.tensor.matmul(Ghh[:, g * BL:(g + 1) * BL],
                                     W_comb[DC + k][:, g * 128:(g + 1) * 128],
                                     h_bf[:, k * BL:(k + 1) * BL],
                                     start=(k == 0), stop=(k == HC - 1),
                                     skip_group_check=True)

            # ---- masked softmax straight from PSUM ----
            nmx = loop_sb.tile([BL, 1], F32, name="nmx", tag="nmx")
            nc.vector.tensor_reduce(nmx[:], Eps4[0:BL, :], mybir.AxisListType.X,
                                    ALU.max, negate=True)
            nc.scalar.activation(En_sb[:], Eps4[0:BL, :], AF.Exp, bias=nmx[:],
                                 scale=1.0)
            se = loop_sb.tile([BL, 1], F32, name="se", tag="se")
            nc.vector.scalar_tensor_tensor(En_sb[:], En_sb[:], 1.0, m01[:],
                                           ALU.mult, ALU.mult, accum_out=se[:])
            rse = loop_sb.tile([BL, 1], F32, name="rse", tag="rse")
            nc.vector.reciprocal(rse[:], se[:])
            nc.vector.tensor_scalar(w_att_n[:], En_sb[:], rse[:], None, ALU.mult)

            # accum += w_att_n * finv * 0.5 ; broadcast q for next step
            nc.vector.tensor_tensor(En_sb[:], w_att_n[:], finv05[:], ALU.mult)
            nc.vector.tensor_tensor(accum[:], accum[:], En_sb[:], ALU.add)
            nc.vector.tensor_copy(accum_bf[:], accum[:])

            # transpose w_att_n chunks -> [128, (tc, b)]
            wtp = loop_ps.tile([128, TCmax * BL], BF16, name="wtp", tag="wtp")
            for t in range(TCmax):
                nc.tensor.transpose(wtp[:, t * BL:(t + 1) * BL],
                                    w_att_n[0:BL, t * 128:(t + 1) * 128],
                                    ident[0:BL, 0:BL])
            wts = loop_sb.tile([128, TCmax * BL], BF16, name="wts", tag="wts")
            nc.vector.tensor_copy(wts[:], wtp[:])

            # ctx: enc-stationary matvecs over the resident enc tiles
            c_ps = loop_ps.tile([128, DC * BL], F32, name="c_ps", tag="c_ps")
            for j in range(BL):
                for dk in range(DC):
                    for t in range(TC[j]):
                        nc.tensor.matmul(
                            c_ps[:, dk * BL + j:dk * BL + j + 1],
                            enc_sb[j][t][:, dk * 128:(dk + 1) * 128],
                            wts[:, t * BL + j:t * BL + j + 1],
                            start=(t == 0), stop=(t == TC[j] - 1))
            nc.vector.tensor_copy(ctxT_sb[:], c_ps[:])
            dst = cstk_d[:, bass.ds(t4, BL)].rearrange("(c p) b -> p c b", p=128)
            nc.sync.dma_start(dst, ctxT_sb[:])

        # ============ READOUT ============
        inner.close()
        post_sb = ctx.enter_context(tc.tile_pool(name="post_sb", bufs=1))
        post_st = ctx.enter_context(tc.tile_pool(name="post_st", bufs=2))
        post_ps = ctx.enter_context(tc.tile_pool(name="post_ps", bufs=2,
                                                 space="PSUM"))

        # stage xro moving tiles: [s(8) | emb(5) | ctx(4)] chunks of [128, NS]
        xro = []
        for k in range(HC):
            tl = post_sb.tile([128, NS], BF16, name=f"xh{k}", tag=f"xh{k}")
            nc.sync.dma_start(tl[:], hstk_d[k * 128:(k + 1) * 128, :])
            xro.append(tl)
        for k in range(EC):
            tl = post_sb.tile([128, NS], BF16, name=f"xe{k}", tag=f"xe{k}")
            nc.sync.dma_start(tl[:], embT_d[k * 128:(k + 1) * 128, :])
            xro.append(tl)
        for k in range(DC):
            tl = post_sb.tile([128, NS], BF16, name=f"xc{k}", tag=f"xc{k}")
            nc.sync.dma_start(tl[:], cstk_d[k * 128:(k + 1) * 128, :])
            xro.append(tl)
        W_roe = [post_sb.tile([128, RO // 2], BF16, name=f"wre{k}", tag=f"wre{k}")
                 for k in range(XROC)]
        W_roo = [post_sb.tile([128, RO // 2], BF16, name=f"wro{k}", tag=f"wro{k}")
                 for k in range(XROC)]
        for k in range(XROC):
            nc.sync.dma_start(W_roe[k][:], W_roT_e_d[k * 128:(k + 1) * 128, :])
            nc.sync.dma_start(W_roo[k][:], W_roT_o_d[k * 128:(k + 1) * 128, :])
        pe_touch(xro[0][:, 0:128])
        pe_touch(W_roe[0][:, 0:128])
        pe_touch(W_roo[0][:, 0:128])

        maxo = []
        for oc in range(ROC):
            Re = post_ps.tile([128, NS], F32, name="Re", tag="Re")
            for k in range(XROC):
                nc.tensor.matmul(Re[:], W_roe[k][:, oc * 128:(oc + 1) * 128],
                                 xro[k][:], start=(k == 0), stop=(k == XROC - 1))
            t1 = post_st.tile([128, NS], F32, name="t1", tag="t1")
            nc.scalar.activation(t1[:], Re[:], AF.Identity,
                                 bias=b_ro_e[:, oc:oc + 1], scale=1.0)
            Ro = post_ps.tile([128, NS], F32, name="Ro", tag="Re")
            for k in range(XROC):
                nc.tensor.matmul(Ro[:], W_roo[k][:, oc * 128:(oc + 1) * 128],
                                 xro[k][:], start=(k == 0), stop=(k == XROC - 1))
            t2 = post_st.tile([128, NS], F32, name="t2", tag="t2")
            nc.scalar.activation(t2[:], Ro[:], AF.Identity,
                                 bias=b_ro_o[:, oc:oc + 1], scale=1.0)
            mo = post_sb.tile([128, NS], BF16, name=f"mo{oc}", tag=f"mo{oc}")
            nc.vector.tensor_tensor(mo[:], t1[:], t2[:], ALU.max)
            maxo.append(mo)

        # logits
        wo_pool = ctx.enter_context(tc.tile_pool(name="wo", bufs=6))
        first = True
        for vc in range(VC):
            wo = [wo_pool.tile([128, 128], BF16, name=f"wo{vc}_{k}", tag=f"wok{k}")
                  for k in range(ROC)]
            for k in range(ROC):
                nc.sync.dma_start(wo[k][:],
                                  W_outT_d[k * 128:(k + 1) * 128,
                                           vc * 128:(vc + 1) * 128])
            if first:
                pe_touch(wo[0][:, 0:128])
                pe_touch(maxo[0][:, 0:128])
                first = False
            L = post_ps.tile([128, NS], F32, name="L", tag="L")
            for k in range(ROC):
                nc.tensor.matmul(L[:], wo[k][:], maxo[k][:],
                                 start=(k == 0), stop=(k == ROC - 1))
            lo = post_st.tile([128, NS], F32, name="lo", tag="lo")
            nc.scalar.activation(lo[:], L[:], AF.Identity,
                                 bias=b_out_col[:, vc:vc + 1], scale=1.0)
            nc.sync.dma_start(out_d[vc * 128:(vc + 1) * 128, :], lo[:])

    return nc


def check_waits(nc, cap_note=""):
    """Print compute instructions with >=2 sync waits (walrus limit is 1)."""
    bad = []
    for fn in nc.m.functions:
        for bb in fn.blocks:
            for inst in bb.instructions:
                c = inst.concise()
                nw = c.count("wait:")
                eng = c.split()[0] if c.split() else "?"
                if nw >= 2 and eng in ("PE", "ACT", "DVE", "PL"):
                    bad.append((nw, c[:180]))
    for nw, c in bad:
        print("WAITS", nw, c)
    return bad


def _make_vT4(v_att):
    """[128, (a, j, 4)]: col (a,j,j2) = v chunk a if j2 == j else 0."""
    v = v_att.reshape(AC, 128)  # [a, 128]
    out = np.zeros((128, AC, BL, BL), np.float32)
    for a in range(AC):
        for j in range(BL):
            out[:, a, j, j] = v[a]
    return out.reshape(128, AC * BL * BL)


def _prep_core(inputs, order, Tp, core):
    """Host-side data prep for one core. Returns the in_map dict."""
    enc = np.asarray(inputs["encoder_outputs"], np.float32)
    labels = np.asarray(inputs["labels"])
    lens = np.asarray(inputs["enc_seq_len"], np.int64)
    embed = np.asarray(inputs["embed"], np.float32)

    bidx = [int(order[j * NCORE + core]) for j in range(BL)]
    m = {}
    for j in range(BL):
        b = bidx[j]
        ep = np.zeros((Tp[j], D), np.float32)
        ep[:T] = enc[b, :Tp[j] if Tp[j] <= T else T]
        m[f"enc_td{j}"] = _bf(ep)
        m[f"encT{j}"] = _bf(ep.T)
    # shifted embeddings: [S, BL, E] -> embT [E, (s, j)]
    emb = np.zeros((BL, S, E), np.float32)
    for j in range(BL):
        b = bidx[j]
        emb[j, 1:] = embed[labels[b, :S - 1].astype(np.int64)]
    embT = emb.transpose(2, 1, 0).reshape(E, NS)  # (E, (s, j))
    m["embT"] = _bf(embT)
    # multiplicative mask (1 inside the sequence, 0 outside)
    m01 = np.zeros((BL, 1024), np.float32)
    for j in range(BL):
        m01[j, :int(lens[bidx[j]])] = 1.0
    m["m01"] = _bf(m01)
    return m, bidx


def kernel(**inputs):
    lens = np.asarray(inputs["enc_seq_len"], np.int64)
    order = np.argsort(-lens, kind="stable")
    Tp = []
    for j in range(BL):
        mx = max(int(lens[order[j * NCORE + i]]) for i in range(NCORE))
        Tp.append(min(1024, ((mx + 127) // 128) * 128))

    W_ih = np.asarray(inputs["W_ih"], np.float32)
    W_hh = np.asarray(inputs["W_hh"], np.float32)
    shared = {
        "W_combT": _bf(np.concatenate([W_ih[:, E:].T, W_hh.T], 0)),
        "W_ih_embT": _bf(W_ih[:, :E].T),
        "W_encT": _bf(np.asarray(inputs["W_enc"], np.float32).T),
        "W_sT": _bf(np.asarray(inputs["W_s"], np.float32).T),
        "wfert_col": _bf(np.asarray(inputs["W_fert"],
                                    np.float32).reshape(DC, 128).T),
        "vT_col": _bf(np.asarray(inputs["v_att"], np.float32).reshape(AC, 128).T),
        "vT4": _bf(_make_vT4(np.asarray(inputs["v_att"], np.float32))),
        "wfb_col": _bf(np.asarray(inputs["W_fb"], np.float32)[:, 0]
                       .reshape(AC, 128).T),
        "b_enc_col": np.ascontiguousarray(
            np.asarray(inputs["b_enc"], np.float32).reshape(AC, 128).T),
        "b_comb": np.ascontiguousarray(
            (np.asarray(inputs["b_ih"], np.float32)
             + np.asarray(inputs["b_hh"], np.float32)).reshape(GC, 128).T),
        "W_roT_e": _bf(np.asarray(inputs["W_ro"], np.float32)[0::2].T),
        "W_roT_o": _bf(np.asarray(inputs["W_ro"], np.float32)[1::2].T),
        "b_ro_e": np.ascontiguousarray(
            np.asarray(inputs["b_ro"], np.float32)[0::2].reshape(ROC, 128).T),
        "b_ro_o": np.ascontiguousarray(
            np.asarray(inputs["b_ro"], np.float32)[1::2].reshape(ROC, 128).T),
        "W_outT": _bf(np.asarray(inputs["W_out"], np.float32).T),
        "b_out_col": np.ascontiguousarray(
            np.asarray(inputs["b_out"], np.float32).reshape(VC, 128).T),
    }

    in_maps = []
    bidx_all = []
    for c in range(NCORE):
        m, bidx = _prep_core(inputs, order, Tp, c)
        m.update(shared)
        in_maps.append(m)
        bidx_all.append(bidx)

    nc = build_nc(Tp)
    nc.finalize()
    from concourse.bass_utils import run_bass_kernel_spmd
    import os as _os
    trace = bool(_os.environ.get("BASS_KERNEL_TRACE"))
    res = run_bass_kernel_spmd(nc, in_maps, core_ids=list(range(NCORE)),
                               trace=trace)
    global LAST_EXEC_NS, LAST_OUTS, LAST_META
    LAST_EXEC_NS = res.exec_time_ns
    outs = res.results
    LAST_OUTS = outs
    LAST_META = (order, Tp, bidx_all)

    logits = np.zeros((B, S, V), np.float32)
    for c in range(NCORE):
        o = outs[c]["out"].reshape(V, S, BL)
        for j in range(BL):
            logits[bidx_all[c][j]] = o[:, :, j].T
    return logits


if __name__ == "__main__":
    # quick build + wait check
    nc = build_nc([1024, 896, 768, 640])
    bad = check_waits(nc)
    print(f"{len(bad)} instructions with >=2 waits")
